# revision 1
# baseline (speedup 1.0000x reference)
"""Trainium2 Bass kernel for nn_CrossAttention (B=4, S=1024, D=512, H=8).

Sharding: 8 cores = batch (4) x head-group (2 groups of 4 heads).
Each core computes a partial [S, E] output over its 256 feature dims;
the host sums the two partials per batch and adds the bias.

Per-core math (feature-major / transposed activation layout):
  allT  [512, 2049] = [l2rT | r2lT | tembT]           (k order: l2r, r2l, temb)
  kvT   [256, 2049] = Wk_slice @ allT                 (shared q/k/v projection)
  qT    [256, 1024] = kvT[:, :1024] + kvT[:, 1024:2048]   (linearity of proj)
  per head h (hd=64), per 512-query tile, per visible 128-k block:
    logitsT [128k, 512q] = kvT_h_blk.T-contract @ qT_h    (PE, K=64)
    expT = exp(0.125 * logitsT)                           (ACT, reads PSUM)
    triangular/pad masks applied with in-place affine_select(fill=0)
    xT [65, 512] += kv_aug_blk.T-contract @ expT          (kv_aug has ones col
                                                           -> row 64 = denom)
  normalize via reciprocal + PE ones-broadcast, then
  out_part [1024, 512] = xT.T @ Wo_slice  (accumulated over 4 heads)
"""

import sys

sys.path.insert(0, "/opt/trn_rl_repo")

from contextlib import ExitStack

import numpy as np

import concourse.bass as bass
import concourse.mybir as mybir
import concourse.tile as tile
from concourse import bacc
from concourse.bass import ds, ts
from concourse.bass_utils import run_bass_kernel_spmd
from concourse.masks import make_identity


def _ensure_ntff_hook():
    """This image's antenv lacks axon_hooks; synthesize it so trace=True can
    reach the libaxon NTFF profiler (used by test.py, harmless otherwise)."""
    import types

    try:
        from antenv import axon_hooks  # noqa: F401

        return
    except ImportError:
        pass
    mod = types.ModuleType("antenv.axon_hooks")
    mod._hook = None
    mod.set_axon_ntff_profile_hook = lambda h: setattr(mod, "_hook", h)
    mod.get_axon_ntff_profile_hook = lambda: mod._hook
    import antenv

    sys.modules["antenv.axon_hooks"] = mod
    antenv.axon_hooks = mod
    try:
        from trn_agent_boot.trn_boot import _ntff_profile_via_ctypes

        mod._hook = _ntff_profile_via_ctypes("/opt/axon/libaxon_pjrt.so")
    except Exception:
        pass


_ensure_ntff_hook()


def _enable_ldw_opt():
    """Flip walrus's --enable-ldw-opt (hardcoded false in bass_utils): with
    one LDWEIGHTS per matmul serialized against its MM, ~80us of the PE span
    is weight loads. Opt-out via KERNEL_LDW_OPT=0."""
    import os

    if os.environ.get("KERNEL_LDW_OPT", "0") != "1":
        return
    import concourse.bass_utils as _bu

    orig = _bu.run_command

    def patched(argv, **kwargs):
        argv = [
            "--enable-ldw-opt=true" if a == "--enable-ldw-opt=false" else a
            for a in argv
        ]
        return orig(argv, **kwargs)

    if getattr(_bu.run_command, "_ldw_patched", None) is None:
        patched._ldw_patched = True
        _bu.run_command = patched


_enable_ldw_opt()

F32 = mybir.dt.float32
F32R = mybir.dt.float32r
BF16 = mybir.dt.bfloat16
AF = mybir.ActivationFunctionType
ALU = mybir.AluOpType

P = 128
S = 1024
D = 512
E = 512
HG = 4  # heads per core
HD = 64
CS = HG * HD  # 256 feature cols per core
NKB = 17  # padded k blocks: 8 l2r + 8 r2l + 1 (temb+pad)
KL = NKB * P  # 2176
KV_REAL = 2 * S + 1  # 2049


def _r(ap):
    return ap.bitcast(F32R)


def _visible_blocks(q0):
    """k blocks visible to query tile [q0, q0+512); (kb, mask) with mask None
    for fully-visible, else (kind, d)."""
    vis = []
    for kb in range(8):  # l2r keys: visible iff q >= j
        d = 128 * kb - q0
        if d >= 512:
            continue  # fully masked
        vis.append((kb, None if d <= -128 else ("l2r", d)))
    for kbl in range(8):  # r2l keys: visible iff q <= j
        d = 128 * kbl - q0
        if d <= -128:
            continue  # fully masked
        vis.append((8 + kbl, None if d >= 511 else ("r2l", d)))
    vis.append((16, ("temb", 0)))  # partition 0 = temb col, rest = pad
    return vis


def _build_body(ctx, tc):
    nc = tc.nc
    ctx.enter_context(
        nc.allow_low_precision(reason="f32r rounding discipline for PE matmuls")
    )

    xlT = nc.dram_tensor("xlT", [D, S], F32, kind="ExternalInput").ap()
    xrT = nc.dram_tensor("xrT", [D, S], F32, kind="ExternalInput").ap()
    tembT = nc.dram_tensor("tembT", [D, 1], F32, kind="ExternalInput").ap()
    wkT = nc.dram_tensor("wkT", [D, CS], F32, kind="ExternalInput").ap()
    woT = nc.dram_tensor("woT", [CS, E], F32, kind="ExternalInput").ap()
    out = nc.dram_tensor("out_part", [S, E], F32, kind="ExternalOutput").ap()

    const = ctx.enter_context(tc.tile_pool(name="const", bufs=1))
    inp = ctx.enter_context(tc.tile_pool(name="inp", bufs=1))
    kvp = ctx.enter_context(tc.tile_pool(name="kvp", bufs=1))
    kvag = ctx.enter_context(tc.tile_pool(name="kvag", bufs=1))
    xts = ctx.enter_context(tc.tile_pool(name="xts", bufs=1))
    expp = ctx.enter_context(tc.tile_pool(name="expp", bufs=30))
    sres = ctx.enter_context(tc.tile_pool(name="sres", bufs=4))
    outp = ctx.enter_context(tc.tile_pool(name="outp", bufs=3))
    ps512 = ctx.enter_context(tc.tile_pool(name="ps512", bufs=3, space="PSUM"))
    psX = ctx.enter_context(tc.tile_pool(name="psX", bufs=4, space="PSUM"))
    psC = ctx.enter_context(tc.tile_pool(name="psC", bufs=1, space="PSUM"))

    ident = const.tile([P, P], BF16)
    ident_stage = const.tile([P, P], F32)
    make_identity(nc, ident_stage[:])
    nc.vector.tensor_copy(ident[:], ident_stage[:])  # cast to bf16
    ones = const.tile([65, HD], F32)
    ones_stage = const.tile([65, HD], F32)
    nc.gpsimd.memset(ones_stage[:], 1.0)
    nc.vector.tensor_copy(_r(ones[:]), ones_stage[:])  # round to f32r

    # ---- input DMAs ----
    # f32r matmul operands must be *written* by a rounding compute op, so DMA
    # lands in staging tiles and ACT/DVE round-copies into the real tiles.
    stg = ctx.enter_context(tc.tile_pool(name="stg", bufs=4))
    allT = [inp.tile([P, KV_REAL + 1], F32, name=f"allT{j}") for j in range(4)]
    for j in range(4):
        for src_ap, c0, w in (
            (xlT[ts(j, P), :], 0, S),
            (xrT[ts(j, P), :], S, S),
            (tembT[ts(j, P), :], 2 * S, 1),
        ):
            st = stg.tile([P, 1024], F32, name="st", tag="st")
            nc.sync.dma_start(out=st[:, 0:w], in_=src_ap)
            if w == 1:  # fp32r matmul needs even free counts: add a zero col
                nc.vector.memset(st[:, 1:2], 0.0)
                w = 2
            nc.scalar.activation(_r(allT[j][:, ds(c0, w)]), st[:, 0:w], AF.Copy)
    wk = inp.tile([P, 4, CS], F32)
    st = stg.tile([P, 1024], F32, name="st", tag="st")
    nc.sync.dma_start(
        out=st[:].rearrange("p (c n) -> p c n", c=4),
        in_=wkT.rearrange("(c p) n -> p c n", p=P),
    )
    nc.vector.tensor_copy(_r(wk[:]), st[:].rearrange("p (c n) -> p c n", c=4))
    wo = inp.tile([HD, HG, E], F32)
    for half in range(2):
        st = stg.tile([P, 1024], F32, name="st", tag="st")
        nc.sync.dma_start(
            out=st[0:HD, :].rearrange("p (c n) -> p c n", c=2),
            in_=woT.rearrange("(g p) n -> p g n", p=HD)[:, ds(half * 2, 2), :],
        )
        nc.vector.tensor_copy(
            _r(wo[:, ds(half * 2, 2), :]),
            st[0:HD, :].rearrange("p (c n) -> p c n", c=2),
        )

    # ---- shared qkv projection: kvT[c][128, KL], c-chunks of 128 ----
    kvT = [kvp.tile([P, KL], BF16, name=f"kvT{c}") for c in range(2)]
    qT = [kvp.tile([P, S], BF16, name=f"qT{c}") for c in range(2)]
    ntiles = [(0, 512), (512, 512), (1024, 512), (1536, 512), (2048, 2)]
    zst = stg.tile([P, 1024], F32, name="zst", tag="st")
    nc.vector.memset(zst[:], 0.0)
    for c in range(2):
        # zero pad cols via rounded copy (f32r memset fails ISA check)
        nc.vector.tensor_copy(kvT[c][:, KV_REAL:KL], zst[:, 0 : KL - KV_REAL])
        for n0, nw in ntiles:
            pp = ps512.tile([P, 512], F32, name="pp", tag="ps")
            for j in range(4):
                nc.tensor.matmul(
                    pp[:, 0:nw],
                    _r(wk[:, j, ts(c, P)]),
                    _r(allT[j][:, ds(n0, nw)]),
                    start=(j == 0),
                    stop=(j == 3),
                )
            nc.vector.tensor_copy(kvT[c][:, ds(n0, nw)], pp[:, 0:nw])
        nc.vector.tensor_add(qT[c][:], kvT[c][:, 0:S], kvT[c][:, S : 2 * S])

    # ---- kv in natural [k, d] layout, ones-augmented: kva[h][128, 17, 65] ----
    kva = [kvag.tile([P, NKB, 65], BF16, name=f"kva{h}") for h in range(HG)]
    ost = stg.tile([P, NKB], F32, name="ost", tag="st")
    nc.vector.memset(ost[:], 1.0)
    for h in range(HG):
        c, ho = h // 2, 64 * (h % 2)
        # only the ones-columns need init; transposes fill cols 0..63
        nc.vector.tensor_copy(
            kva[h][:, :, 64:65],
            ost[:].rearrange("p (a b) -> p a b", b=1),
        )
        for g, nblk in ((0, 8), (1, 8), (2, 1)):
            tp = ps512.tile([P, 8, HD], BF16, name="tp", tag="ps")
            for b in range(nblk):
                kb = g * 8 + b
                nc.tensor.transpose(
                    tp[:, b, :],
                    kvT[c][ho : ho + HD, ts(kb, P)],
                    ident[ho : ho + HD, ho : ho + HD],
                )
            nc.vector.tensor_copy(
                kva[h][:, ds(g * 8, nblk), 0:HD], tp[:, 0:nblk, :]
            )

    # ---- precomputed 0/1 bf16 mask tiles (DVE mul is ~7x cheaper than a
    # per-block gpsimd affine_select) ----
    maskp = ctx.enter_context(tc.tile_pool(name="maskp", bufs=1))
    ones_bf = maskp.tile([P, 512], BF16)
    nc.gpsimd.memset(ones_bf[:], 1.0)
    masks = {}
    for d in (0, 128, 256, 384):
        mt = maskp.tile([P, 512], BF16, name=f"ml2r{d}")
        nc.gpsimd.affine_select(
            mt[:], ones_bf[:], pattern=[[1, 512]], compare_op=ALU.is_ge,
            fill=0.0, base=-d, channel_multiplier=-1,
        )
        masks[("l2r", d)] = mt
        mt = maskp.tile([P, 512], BF16, name=f"mr2l{d}")
        nc.gpsimd.affine_select(
            mt[:], ones_bf[:], pattern=[[-1, 512]], compare_op=ALU.is_ge,
            fill=0.0, base=d, channel_multiplier=1,
        )
        masks[("r2l", d)] = mt
    mt = maskp.tile([P, 512], BF16, name="mtemb")
    nc.gpsimd.affine_select(
        mt[:], ones_bf[:], pattern=[[0, 512]], compare_op=ALU.is_ge,
        fill=0.0, base=0, channel_multiplier=-1,
    )
    masks[("temb", 0)] = mt

    # ---- attention ----
    # Emit all logits MMs of a stream before its AV MMs: PE is in-order, so
    # interleaving lg/av stalls PE on the ACT exp + gpsimd mask chain (and the
    # stalls keep HAM cold, halving the PE clock).
    xt = [xts.tile([HD, S], F32, name=f"xt{h}") for h in range(HG)]
    for c in range(2):
        hpair = (2 * c, 2 * c + 1)
        for qi in range(2):
            q0 = qi * 512
            vis = _visible_blocks(q0)
            xps = {h: psX.tile([65, 512], F32, name=f"xps{h % 2}", tag="xps") for h in hpair}
            exs = []
            for kb, mask in vis:
                exh = {}
                for h in hpair:
                    ho = 64 * (h % 2)
                    lg = ps512.tile([P, 512], F32, name="lg", tag="ps")
                    nc.tensor.matmul(
                        lg[:],
                        kvT[c][ho : ho + HD, ts(kb, P)],
                        qT[c][ho : ho + HD, ds(q0, 512)],
                        start=True,
                        stop=True,
                    )
                    ex = expp.tile([P, 512], BF16, name="ex")
                    nc.scalar.activation(ex[:], lg[:], AF.Exp, scale=0.125)
                    if mask is not None:
                        nc.vector.tensor_mul(ex[:], ex[:], masks[mask][:])
                    exh[h] = ex
                exs.append((kb, exh))
            for i, (kb, exh) in enumerate(exs):
                for h in hpair:
                    nc.tensor.matmul(
                        xps[h][:],
                        kva[h][:, kb, :],
                        exh[h][:],
                        start=(i == 0),
                        stop=(i == len(exs) - 1),
                    )
            for h in hpair:
                cs = sres.tile([65, 512], F32, name="cs")
                nc.vector.tensor_copy(_r(cs[64:65, :]), xps[h][64:65, :])
                bc = psC.tile([HD, 512], F32, name="bc")
                nc.tensor.matmul(
                    bc[:], _r(ones[64:65, :]), _r(cs[64:65, :]),
                    start=True, stop=True,
                )
                bcs = sres.tile([HD, 512], F32, name="bcs")
                nc.vector.reciprocal_approx_fast(bcs[:], bc[:])
                nc.vector.tensor_mul(
                    _r(xt[h][:, ds(q0, 512)]), xps[h][0:HD, :], bcs[:]
                )

    # ---- output projection: out[s, e] += xt[h].T @ wo_h ----
    for st in range(8):
        pf = ps512.tile([P, E], F32, name="pf", tag="ps")
        for h in range(HG):
            nc.tensor.matmul(
                pf[:],
                _r(xt[h][:, ts(st, P)]),
                _r(wo[:, h, :]),
                start=(h == 0),
                stop=(h == 3),
            )
        ob = outp.tile([P, E], F32, name="ob")
        nc.vector.tensor_copy(ob[:], pf[:])
        nc.sync.dma_start(out=out[ts(st, P), :], in_=ob[:])


_NC_CACHE = None


def build_nc():
    global _NC_CACHE
    if _NC_CACHE is None:
        nc = bacc.Bacc(
            "TRN2",
            target_bir_lowering=False,
            debug=False,
            num_devices=8,
        )
        with tile.TileContext(nc) as tc, ExitStack() as ctx:
            _build_body(ctx, tc)
        nc.compile()
        _NC_CACHE = nc
    return _NC_CACHE


def make_in_maps(l2r_embed, r2l_embed, temb, W_dense, W_out):
    in_maps = []
    for core in range(8):
        b, hg = core // 2, core % 2
        cols = slice(CS * hg, CS * (hg + 1))
        in_maps.append(
            {
                "xlT": np.ascontiguousarray(l2r_embed[b].T),
                "xrT": np.ascontiguousarray(r2l_embed[b].T),
                "tembT": np.ascontiguousarray(temb[b][:, None]),
                "wkT": np.ascontiguousarray(W_dense[cols, :].T),
                "woT": np.ascontiguousarray(W_out[:, cols].T),
            }
        )
    return in_maps


def kernel(l2r_embed, r2l_embed, temb, W_dense, W_out, b_out, num_heads, **run_kwargs):
    assert int(num_heads) == 8
    l2r_embed = np.asarray(l2r_embed, np.float32)
    r2l_embed = np.asarray(r2l_embed, np.float32)
    temb = np.asarray(temb, np.float32)
    W_dense = np.asarray(W_dense, np.float32)
    W_out = np.asarray(W_out, np.float32)
    b_out = np.asarray(b_out, np.float32)

    nc = build_nc()
    in_maps = make_in_maps(l2r_embed, r2l_embed, temb, W_dense, W_out)
    res = run_bass_kernel_spmd(nc, in_maps, core_ids=list(range(8)), **run_kwargs)

    B = l2r_embed.shape[0]
    outp = np.empty((B, S, E), np.float32)
    for b in range(B):
        outp[b] = (
            res.results[2 * b]["out_part"]
            + res.results[2 * b + 1]["out_part"]
            + b_out[None, :]
        )
    if run_kwargs:
        kernel.last_results = res
    return outp



# revision 14
# speedup vs baseline: 1.1968x; 1.1968x over previous
"""Trainium2 Bass kernel for nn_CrossAttention (B=4, S=1024, D=512, H=8).

Sharding: 8 cores = batch (4) x head-group (2 groups of 4 heads).
Each core computes a partial [S, E] output over its 256 feature dims;
the host sums the two partials per batch and adds the bias.

v2 design (vs f32r baseline): bf16 end-to-end.
  - Host converts inputs to bf16 -> half the DMA bytes, no staging
    round-copies, FWL weight loads, 1 cyc/col matmul streaming.
  - kvT [128, 2176] per 128-feature chunk c: shared qkv projection
    (k order: l2r 0..1023 | r2l 1024..2047 | temb 2048 | zero pad).
    qT = kvT[:, :1024] + kvT[:, 1024:2048]  (linearity).
  - kva [128k, 17kb, 4h, 65] (feats + ones col) built by DMA-engine
    transposes of kvT 128x128 blocks (zero PE/DVE cost).  ones col is 0
    for pad k rows so pad keys contribute nothing to AV or denominator.
  - Attention in 16 streams (c, qtile 256, head-in-pair): 11 visible
    k-blocks -> logits psum split 6+5 blocks (3+3 banks, bufs=2), one
    wide exp ACTIVATE per psum tile (amortizes the ~293ns ACT fixed
    cost), single [128,1024] bf16 diag-mask multiply (the 4 masked
    blocks are always contiguous at stream cols [512t, 512t+1024)),
    11 AV matmuls accumulate [65, 256] (row 64 = denominator via ones).
  - normalize: DVE reciprocal of denom row, gpsimd partition_broadcast,
    DVE multiply into per-head xt [64, 1024] bf16.
  - out projection: per 128-row s-block, 4 accumulating K=64 matmuls
    (one per head) -> psum -> copy (DVE/ACT alternating) -> DMA f32.
"""

import sys

sys.path.insert(0, "/opt/trn_rl_repo")

from contextlib import ExitStack

import ml_dtypes
import numpy as np

import concourse.bass as bass
import concourse.mybir as mybir
import concourse.tile as tile
from concourse import bacc
from concourse.bass import ds, ts
from concourse.bass_utils import run_bass_kernel_spmd


def _ensure_ntff_hook():
    """This image's antenv lacks axon_hooks; synthesize it so trace=True can
    reach the libaxon NTFF profiler (used by test.py, harmless otherwise)."""
    import types

    try:
        from antenv import axon_hooks  # noqa: F401

        return
    except ImportError:
        pass
    mod = types.ModuleType("antenv.axon_hooks")
    mod._hook = None
    mod.set_axon_ntff_profile_hook = lambda h: setattr(mod, "_hook", h)
    mod.get_axon_ntff_profile_hook = lambda: mod._hook
    import antenv

    sys.modules["antenv.axon_hooks"] = mod
    antenv.axon_hooks = mod
    try:
        from trn_agent_boot.trn_boot import _ntff_profile_via_ctypes

        mod._hook = _ntff_profile_via_ctypes("/opt/axon/libaxon_pjrt.so")
    except Exception:
        pass


_ensure_ntff_hook()

F32 = mybir.dt.float32
BF16 = mybir.dt.bfloat16
AF = mybir.ActivationFunctionType
ALU = mybir.AluOpType

P = 128
S = 1024
D = 512
E = 512
HG = 4  # heads per core
HD = 64
CS = HG * HD  # 256 feature cols per core
NKB = 17  # k blocks: 8 l2r + 8 r2l + 1 (temb+pad)
KL = NKB * P  # 2176
QW = 256  # query tile width
NBLK = 11  # visible k blocks per stream
SPLIT = 6  # blocks in the first logits psum tile (rest in the second)
EXW = NBLK * QW  # 2816


def _vis(t):
    """Visible k-block indices for query tile [256t, 256t+256), in stream
    order: l2r 0..2t+1 (last two diag-masked), r2l 2t..7 (first two
    diag-masked), temb.  The 4 masked blocks sit at stream cols
    [512t, 512t+1024)."""
    return list(range(2 * t + 2)) + list(range(8 + 2 * t, 16)) + [16]


DEBUG = False  # adds intermediate dumps (kvT/qT/kva/ex) as extra outputs


def _r(ap):
    return ap.bitcast(mybir.dt.float32r)


def _build_body(ctx, tc):
    nc = tc.nc
    ctx.enter_context(nc.allow_low_precision(reason="bf16 attention pipeline"))

    xlT = nc.dram_tensor("xlT", [D, S], BF16, kind="ExternalInput").ap()
    xrT = nc.dram_tensor("xrT", [D, S], BF16, kind="ExternalInput").ap()
    tembT = nc.dram_tensor("tembT", [D, 1], BF16, kind="ExternalInput").ap()
    wkT = nc.dram_tensor("wkT", [D, CS], BF16, kind="ExternalInput").ap()
    woT = nc.dram_tensor("woT", [CS, E], BF16, kind="ExternalInput").ap()
    out = nc.dram_tensor("out_part", [S, E], F32, kind="ExternalOutput").ap()

    inp = ctx.enter_context(tc.tile_pool(name="inp", bufs=1))
    kvp = ctx.enter_context(tc.tile_pool(name="kvp", bufs=1))
    kvag = ctx.enter_context(tc.tile_pool(name="kvag", bufs=1))
    maskp = ctx.enter_context(tc.tile_pool(name="maskp", bufs=1))
    xtp = ctx.enter_context(tc.tile_pool(name="xtp", bufs=1))
    expp = ctx.enter_context(tc.tile_pool(name="expp", bufs=3))
    nrm = ctx.enter_context(tc.tile_pool(name="nrm", bufs=2))
    outp = ctx.enter_context(tc.tile_pool(name="outp", bufs=3))

    # ---- input DMAs (bf16, straight into matmul-ready tiles) ----
    allT = [inp.tile([P, 2052], BF16, name=f"allT{j}") for j in range(4)]
    for j in range(4):
        nc.sync.dma_start(out=allT[j][:, 0:S], in_=xlT[ts(j, P), :])
        nc.sync.dma_start(out=allT[j][:, S : 2 * S], in_=xrT[ts(j, P), :])
        nc.sync.dma_start(out=allT[j][:, 2 * S : 2 * S + 1], in_=tembT[ts(j, P), :])
        nc.vector.memset(allT[j][:, 2 * S + 1 : 2052], 0.0)
    wk = inp.tile([P, 4, CS], BF16)
    nc.sync.dma_start(out=wk[:], in_=wkT.rearrange("(c p) n -> p c n", p=P))
    wo2 = inp.tile([HD, HG, E], BF16)
    nc.sync.dma_start(out=wo2[:], in_=woT.rearrange("(h p) n -> p h n", p=HD))

    # ---- diagonal masks [128k, 4*256q]: l2r d0 | l2r d128 | r2l d0 | r2l d128
    ones_bf = maskp.tile([P, QW], BF16)
    nc.gpsimd.memset(ones_bf[:], 1.0)
    dmask = maskp.tile([P, 4 * QW], BF16)
    for i, (pat, cm, base) in enumerate(
        [(1, -1, 0), (1, -1, -128), (-1, 1, 0), (-1, 1, 128)]
    ):
        nc.gpsimd.affine_select(
            dmask[:, ds(i * QW, QW)],
            ones_bf[:],
            pattern=[[pat, QW]],
            compare_op=ALU.is_ge,
            fill=0.0,
            base=base,
            channel_multiplier=cm,
        )

    # ---- shared qkv projection: kvT[c][128, 2176] ----
    kvT = [kvp.tile([P, KL], BF16, name=f"kvT{c}") for c in range(2)]
    qT = [kvp.tile([P, S], BF16, name=f"qT{c}") for c in range(2)]
    ntiles = [(0, 512), (512, 512), (1024, 512), (1536, 512), (2048, 4)]
    with tc.tile_pool(name="ps512", bufs=3, space="PSUM") as ps512:
        for c in range(2):
            for n0, nw in ntiles:
                pp = ps512.tile([P, 512], F32, name="pp", tag="pp")
                for j in range(4):
                    nc.tensor.matmul(
                        pp[:, 0:nw],
                        wk[:, j, ts(c, P)],
                        allT[j][:, ds(n0, nw)],
                        start=(j == 0),
                        stop=(j == 3),
                    )
                nc.vector.tensor_copy(kvT[c][:, ds(n0, nw)], pp[:, 0:nw])
            nc.vector.memset(kvT[c][:, 2052:KL], 0.0)
            nc.vector.tensor_add(qT[c][:], kvT[c][:, 0:S], kvT[c][:, S : 2 * S])

    # ---- kva [128k, 17, 4h, 128] via DMA-engine transposes of kvT blocks.
    # One transpose per (head, block): the xbar path only honors contiguous
    # 2D output APs with (at least) 256B-aligned offsets, so each head gets
    # a 128-element slot ([feats 0:64 | ones 64 | pad]) written separately.
    KW = P
    kva = kvag.tile([P, NKB, HG, KW], BF16)
    for c in range(2):
        for hh in range(2):
            for kb in range(NKB):
                nc.sync.dma_start_transpose(
                    out=kva[:, kb, 2 * c + hh, 0:HD],
                    in_=kvT[c][ds(HD * hh, HD), ts(kb, P)],
                )
    # ones col: 1 for real keys, 0 for pad rows of the temb block
    nc.vector.memset(kva[:, 0:16, :, HD : HD + 1], 1.0)
    nc.vector.memset(kva[:, 16, :, HD : HD + 1], 0.0)
    nc.vector.memset(kva[0:1, 16, :, HD : HD + 1], 1.0)

    if DEBUG:
        d_kvT = nc.dram_tensor("d_kvT", [2 * P, KL], BF16, kind="ExternalOutput").ap()
        d_qT = nc.dram_tensor("d_qT", [2 * P, S], BF16, kind="ExternalOutput").ap()
        d_kva = nc.dram_tensor(
            "d_kva", [P, NKB * HG * P], BF16, kind="ExternalOutput"
        ).ap()
        for c in range(2):
            nc.sync.dma_start(out=d_kvT[ds(c * P, P), :], in_=kvT[c][:])
            nc.sync.dma_start(out=d_qT[ds(c * P, P), :], in_=qT[c][:])
        nc.sync.dma_start(
            out=d_kva[:],
            in_=kva[:].rearrange("p a b c -> p (a b c)"),
        )

    # ones row (f32r) for the denominator-broadcast matmul
    ones_r = maskp.tile([HD + 1, HD], F32)
    ones_stage = maskp.tile([HD + 1, HD], F32)
    nc.gpsimd.memset(ones_stage[:], 1.0)
    nc.vector.tensor_copy(_r(ones_r[:]), ones_stage[:])

    # ---- attention streams ----
    xt = [xtp.tile([HD, S], BF16, name=f"xt{h}") for h in range(HG)]

    with tc.tile_pool(name="lgp", bufs=2, space="PSUM") as lgp, tc.tile_pool(
        name="xpsp", bufs=2, space="PSUM"
    ) as xpsp:

        if DEBUG:
            d_ex = nc.dram_tensor("d_ex", [P, EXW], BF16, kind="ExternalOutput").ap()
            d_xps = nc.dram_tensor("d_xps", [P, QW], F32, kind="ExternalOutput").ap()
            d_bcs = nc.dram_tensor("d_bcs", [HD, QW], F32, kind="ExternalOutput").ap()
            d_xt = nc.dram_tensor("d_xt", [HG * HD, S], BF16, kind="ExternalOutput").ap()

        def consume(st):
            """AV + normalize for a stream whose exp tile is ready."""
            c, t, hh, ex = st
            h = 2 * c + hh
            vis = _vis(t)
            xps = xpsp.tile([P, 2 * QW], F32, name="xps", tag="xps")
            for j, kbi in enumerate(vis):
                nc.tensor.matmul(
                    xps[0 : HD + 1, 0:QW],
                    kva[:, kbi, h, 0 : HD + 1],
                    ex[:, ds(j * QW, QW)],
                    start=(j == 0),
                    stop=(j == len(vis) - 1),
                )
            # denom row 64 -> SBUF, PE ones-matmul broadcast to 64 rows
            # (gpsimd partition_broadcast reads physical partition 0 on HW,
            # so it cannot broadcast a row living on partition 64)
            cs = nrm.tile([P, QW], F32, name="cs", tag="cs")
            nc.vector.tensor_copy(_r(cs[HD : HD + 1, 0:QW]), xps[HD : HD + 1, 0:QW])
            nc.tensor.matmul(
                xps[0:HD, QW : 2 * QW],
                _r(ones_r[HD : HD + 1, :]),
                _r(cs[HD : HD + 1, 0:QW]),
                start=True,
                stop=True,
            )
            bcs = nrm.tile([HD, QW], F32, name="bcs", tag="bcs")
            nc.vector.reciprocal_approx_fast(bcs[:], xps[0:HD, QW : 2 * QW])
            nc.vector.tensor_mul(
                xt[h][:, ds(t * QW, QW)], xps[0:HD, 0:QW], bcs[:]
            )
            if DEBUG and (c, t, hh) == (0, 0, 0):
                nc.sync.dma_start(out=d_ex[:], in_=ex[:])
                xpc = nrm.tile([P, QW], F32, name="xpc", tag="xpc")
                nc.vector.tensor_copy(xpc[:], xps[:, 0:QW])
                nc.sync.dma_start(out=d_xps[:], in_=xpc[:])
                nc.sync.dma_start(out=d_bcs[:], in_=bcs[:])

        prev = None
        for c in range(2):
            for t in range(4):
                for hh in range(2):
                    ho = HD * hh
                    q0 = t * QW
                    vis = _vis(t)
                    lgA = lgp.tile([P, SPLIT * QW], F32, name="lgA", tag="lg")
                    lgB = lgp.tile([P, SPLIT * QW], F32, name="lgB", tag="lg")
                    ex = expp.tile([P, EXW], BF16, name="ex", tag="ex")
                    for j, kbi in enumerate(vis):
                        dst = (
                            lgA[:, ds(j * QW, QW)]
                            if j < SPLIT
                            else lgB[:, ds((j - SPLIT) * QW, QW)]
                        )
                        nc.tensor.matmul(
                            dst,
                            kvT[c][ho : ho + HD, ts(kbi, P)],
                            qT[c][ho : ho + HD, ds(q0, QW)],
                            start=True,
                            stop=True,
                        )
                    nc.scalar.activation(
                        ex[:, 0 : SPLIT * QW], lgA[:], AF.Exp, scale=0.125
                    )
                    nc.scalar.activation(
                        ex[:, ds(SPLIT * QW, (NBLK - SPLIT) * QW)],
                        lgB[:, 0 : (NBLK - SPLIT) * QW],
                        AF.Exp,
                        scale=0.125,
                    )
                    m0 = 2 * t * QW
                    nc.vector.tensor_mul(
                        ex[:, ds(m0, 4 * QW)], ex[:, ds(m0, 4 * QW)], dmask[:]
                    )
                    if prev is not None:
                        consume(prev)
                    prev = (c, t, hh, ex)
        consume(prev)
        if DEBUG:
            for h in range(HG):
                nc.sync.dma_start(out=d_xt[ds(h * HD, HD), :], in_=xt[h][:])

    # ---- output projection: out[s, e] = sum_h xt[h].T @ wo_h ----
    with tc.tile_pool(name="psF", bufs=2, space="PSUM") as psF:
        for st in range(8):
            pf = psF.tile([P, E], F32, name="pf", tag="pf")
            for h in range(HG):
                nc.tensor.matmul(
                    pf[:],
                    xt[h][:, ts(st, P)],
                    wo2[:, h, :],
                    start=(h == 0),
                    stop=(h == 3),
                )
            ob = outp.tile([P, E], F32, name="ob", tag="ob")
            if st % 2 == 0:
                nc.vector.tensor_copy(ob[:], pf[:])
            else:
                nc.scalar.copy(ob[:], pf[:])
            nc.sync.dma_start(out=out[ts(st, P), :], in_=ob[:])


_NC_CACHE = None


def build_nc():
    global _NC_CACHE
    if _NC_CACHE is None:
        nc = bacc.Bacc(
            "TRN2",
            target_bir_lowering=False,
            debug=False,
            num_devices=8,
        )
        with tile.TileContext(nc) as tc, ExitStack() as ctx:
            _build_body(ctx, tc)
        nc.compile()
        _NC_CACHE = nc
    return _NC_CACHE


def _bf16(x):
    return np.ascontiguousarray(x).astype(ml_dtypes.bfloat16)


def make_in_maps(l2r_embed, r2l_embed, temb, W_dense, W_out):
    in_maps = []
    for core in range(8):
        b, hg = core // 2, core % 2
        cols = slice(CS * hg, CS * (hg + 1))
        in_maps.append(
            {
                "xlT": _bf16(l2r_embed[b].T),
                "xrT": _bf16(r2l_embed[b].T),
                "tembT": _bf16(temb[b][:, None]),
                "wkT": _bf16(W_dense[cols, :].T),
                "woT": _bf16(W_out[:, cols].T),
            }
        )
    return in_maps


def kernel(l2r_embed, r2l_embed, temb, W_dense, W_out, b_out, num_heads, **run_kwargs):
    assert int(num_heads) == 8
    l2r_embed = np.asarray(l2r_embed, np.float32)
    r2l_embed = np.asarray(r2l_embed, np.float32)
    temb = np.asarray(temb, np.float32)
    W_dense = np.asarray(W_dense, np.float32)
    W_out = np.asarray(W_out, np.float32)
    b_out = np.asarray(b_out, np.float32)

    nc = build_nc()
    in_maps = make_in_maps(l2r_embed, r2l_embed, temb, W_dense, W_out)
    res = run_bass_kernel_spmd(nc, in_maps, core_ids=list(range(8)), **run_kwargs)

    B = l2r_embed.shape[0]
    outp = np.empty((B, S, E), np.float32)
    for b in range(B):
        outp[b] = (
            res.results[2 * b]["out_part"]
            + res.results[2 * b + 1]["out_part"]
            + b_out[None, :]
        )
    if run_kwargs:
        kernel.last_results = res
    return outp
